# revision 112
# baseline (speedup 1.0000x reference)
"""MultiHeadAttention (8 heads, d_emb=512, d_hid=64, seq 2048, batch 8) on 8
Trainium2 NeuronCores.

Sharding: data parallel over batch — core i computes batch element i fully
(weights replicated, no collectives).

Per-core pipeline (fp8 everywhere + 4-deep score pipeline; 204.2us
cost-model):
  dtypes:  x^T, Wq/Wk/Wv fp8e4m3 (weights x8 on host -> Q'=8Q etc. sit in
           fp8's sweet spot); scores carry 64x, folded into exp(s'/512);
           concat and Wo also fp8; output stored bf16 (host converts back)
           to halve the tail out-DMA bytes.
  proj:    Q/K/V/Wo matmuls in fp8 DoubleRow (K=256/pass, 0.5 cyc/col);
           V bias via rank-1 fp8 matmul; Q/K bias fused into the eviction.
  startup: XT loaded in column-halves (cc0 gates the first scores); the
           serialized HWDGE generations pace the start, so XT1a-3a/bqk gen
           on the Pool SWDGE channel and every Q/K band-shuffle splits its
           4 DMAs 2/2 across the SP-HWDGE and Pool-SWDGE channels; the
           first K eviction ships a 128-col mini piece, and block 0's
           first four score tiles run as plain k=64 fp8 matmuls straight
           off the eviction staging (h0's dims sit on partitions 0-63),
           skipping the band-shuffle wait entirely; PE+ACT warmed during
           the DMA wait (p-state / Exp table load).
  blocks:  one (head, query-half) per block; the ctx accumulator BORROWS a
           score slot (same pool tag), so the t-loop runs a FOUR-deep
           score-slot rotation (8 PSUM banks); one exp chunk per t split
           144/112 between ScalarE (hw Exp, 1038ns) and VectorE
           (Schraudolph: int8 affine of the score IS the fp8 bit pattern of
           exp, 1192ns) — both exp engines are the global bottleneck and
           run ~85% busy (ACT/DVE are the only engines that can read PSUM,
           so all 256 exp chunks + every eviction must flow through them).
  ctx:     fp8 DoubleRow over key-tile pairs, drained in three tranches +
           normalize at t0-t3 of the NEXT block so PE's in-order queue
           never camps on the burst; block 15 self-drains at t13-t15.
           V_aug ones column makes row 64 the softmax denominator.
  norm:    cx evicted PSUM->SBUF (ACT/DVE); den row DRAM-bounced into a
           partition broadcast; reciprocal via int32 bit-trick on Pool
           (C - bits, ~5% err, harmless here), Pool multiplies -> CCT fp8.
           The final block skips the bounce: DVE recips the PSUM den row
           (bf16), idle PE broadcasts it via a rank-1 bf16 matmul, DVE
           multiplies (tail critical).
  out:     out = concat^T.T @ Wo fp8 DoubleRow; residual adds x+bo (host);
           sts 0-7 overlap the attention blocks (DVE add + Pool
           center/scale); sts 8-15 run as the tail: PE bf16 eye-matmul
           residual, all 8 stat chains emitted before the evicts (tiny ACT
           rstds must not queue behind 612ns evicts), evicts 7-ACT/1-DVE,
           two sts per out-DMA (HWDGE gens paced the tail).
  LN:      gamma==1/beta==0 detected on host -> identity fast path (drops
           the gamma/beta tensor-tensor ops); general path kept as
           fallback.
"""

import copy
import json
import sys
import types

import numpy as np

for _p in ("/opt/trn_rl_repo", "/root/.axon_site/_ro/trn_rl_repo"):
    if _p not in sys.path:
        sys.path.append(_p)

import concourse.bass as bass
import concourse.library_config as library_config
import concourse.mybir as mybir
import concourse.tile as tile

P = 128
S = 2048  # sequence length
E = 512  # embedding dim
H = 8  # heads
D = 64  # head dim
NP = H // 2  # head pairs
ST = S // P  # seq tiles
ET = E // P  # embedding tiles
LN_EPS = 1e-5
F32 = mybir.dt.float32
BF16 = mybir.dt.bfloat16
FP8 = mybir.dt.float8e4
I8 = mybir.dt.int8
I32 = mybir.dt.int32
AF = mybir.ActivationFunctionType
OP = mybir.AluOpType
PM = mybir.MatmulPerfMode

# scores' = (8Q)(8K)^T = 64*scores; true exp arg = scores/8 = scores'/512
EXP_SCALE = 1.0 / 512.0
# Schraudolph to fp8e4m3 bits: byte = 8*log2(e^(s'/512)) + 7*8
SCH_A = 8.0 / (512.0 * np.log(2.0))
SCH_B = 56.25  # +0.25 splits trunc-vs-round ambiguity of the int convert
# int32 bit-trick reciprocal: bits(1/(8x)) ~= C - bits(x), den in [1.4k,3.2k]
REC_C = 0x7D731000

# per-block t's whose exp goes to ScalarE (rest on VectorE): 144 ACT / 112
# DVE total balances ACT (1038ns/chunk + other) vs DVE (1192ns/chunk +
# other); blocks 13/15 take their 9th ACT chunk at t15 so DVE is free for
# the tail-critical normalize
ACT_TS9 = (0, 2, 4, 6, 8, 10, 12, 14, 9)
ACT_TS10 = (0, 2, 4, 6, 8, 10, 12, 14, 9, 5)
ACT10_BLOCKS = ()


# --------------------------------------------------------------------------
# walrus in this build accepts only ONE sync-wait per instruction; Tile's sem
# assignment can attach several (e.g. the kernel-tail drain). Splitting the
# extra waits onto preceding NoOps on the same engine is semantically
# identical (engine streams execute in order).
def _split_waits(m, max_waits=1):
    for fn in m.get("functions", []):
        for blk in fn.get("blocks", []):
            new_insts = []
            for inst in blk.get("instructions", []):
                sync = inst.get("sync_info") or {}
                ow = sync.get("on_wait") or []
                if len(ow) > max_waits:
                    extra = ow[:-max_waits]
                    inst["sync_info"]["on_wait"] = ow[-max_waits:]
                    for ci in range(0, len(extra), max_waits):
                        nop = copy.deepcopy(inst)
                        nop["name"] = f"{inst['name']}ws{ci}"
                        nop["opcode"] = "NoOp"
                        nop["ins"] = []
                        nop["outs"] = []
                        nop["is_reset_sema"] = False
                        nop["sync_info"] = {
                            "on_update": [],
                            "on_wait": extra[ci : ci + max_waits],
                        }
                        new_insts.append(nop)
                new_insts.append(inst)
            blk["instructions"] = new_insts
    return m


def _patch_to_json(nc):
    orig = nc.to_json_bytes

    def patched(self):
        return json.dumps(_split_waits(json.loads(orig()))).encode()

    nc.to_json_bytes = types.MethodType(patched, nc)


def _bcast_ap(ap, parts):
    """[N]-shaped DRAM AP -> [parts, N] via zero-stride partition dim."""
    return bass.AP(
        tensor=ap.tensor, offset=ap.offset, ap=[[0, parts]] + list(ap.ap[-1:])
    )


# --------------------------------------------------------------------------
def build_nc(identity_ln=True):
    nc = bass.Bass()
    xD = nc.declare_dram_parameter("xpb", [S, E], F32, isOutput=False)
    gammaD = nc.declare_dram_parameter("gamma", [E], F32, isOutput=False)
    betaD = nc.declare_dram_parameter("beta", [E], F32, isOutput=False)
    # host-preprocessed layouts: x^T and e-major weights (x8), fp8e4m3
    xTD = nc.declare_dram_parameter("xT", [E, S], FP8, isOutput=False)
    wqpD = nc.declare_dram_parameter("Wq_p", [E, H * D], FP8, isOutput=False)
    wkpD = nc.declare_dram_parameter("Wk_p", [E, H * D], FP8, isOutput=False)
    wvpD = nc.declare_dram_parameter("Wv_p", [E, H * D], FP8, isOutput=False)
    wopD = nc.declare_dram_parameter("Wo_p", [H * D, E], FP8, isOutput=False)
    bqkD = nc.declare_dram_parameter("bqk", [P, 2, NP], F32, isOutput=False)
    bv8D = nc.declare_dram_parameter("bv8", [1, H * D], FP8, isOutput=False)
    eyeD = nc.declare_dram_parameter("eye", [P, P], BF16, isOutput=False)
    xbfD = nc.declare_dram_parameter("xpb_bf", [S, E], BF16, isOutput=False)
    # bf16 output (halved out-DMA bytes; host converts back to f32 —
    # +-0.4% on the final value, well inside the error budget)
    outD = nc.declare_dram_parameter("out", [S, E], BF16, isOutput=True)

    with tile.TileContext(nc) as tc:
        with (
            tc.tile_pool(name="persist", bufs=1) as persist,
            tc.tile_pool(name="dramp", bufs=4, space="DRAM") as dramp,
        ):
            X = persist.tile([P, ST // 2, E], F32, name="Xsb")
            XT = persist.tile([P, ET, S], FP8, name="XTsb")
            Wq_sb = persist.tile([P, ET, H * D], FP8, name="Wq_sb")
            Wk_sb = persist.tile([P, ET, H * D], FP8, name="Wk_sb")
            Wv_sb = persist.tile([P, ET, H * D], FP8, name="Wv_sb")
            Wo_sb = persist.tile([P, ET, E], FP8, name="Wo_sb")
            bqk = persist.tile([P, 2, NP], F32, name="bqk")
            bv8 = persist.tile([1, H * D], FP8, name="bv8")
            ones8 = persist.tile([1, P], FP8, name="ones8")
            ones_bf = persist.tile([1, P], BF16, name="ones_bf")
            ones_f32 = persist.tile([1, P], F32, name="ones_f32")
            gamma_bc = persist.tile([P, E], F32, name="gamma_bc")
            beta_bc = persist.tile([P, E], F32, name="beta_bc")
            eye_bf = persist.tile([P, P], BF16, name="eye_bf")
            Xbf = persist.tile([P, 8, E], BF16, name="Xbf")
            # Q/K in DoubleRow-ready layout, packed across partition
            # bands (matmul base partition must be 0/32/64): bands 0/32/64
            # slot 0 hold pairs 0/1/2; band 0 slot 1 holds pair 3. Within a
            # slot: [head-in-pair, d-half plane, seq]; scores then run fp8
            # DoubleRow (K=64 as 2x32) with tile_position row = band
            QTd = persist.tile([P, 2, 2, 2, S], FP8, name="QTd")
            KTd = persist.tile([P, 2, 2, 2, S], FP8, name="KTd")
            # per-(st,h) block padded to D+2 bytes: dual-fp8 Ldweights needs
            # even k-plane stride/offset (s3_lw_dual_fp8_restrictions)
            Vaug = persist.tile([P, ST, H, D + 2], FP8, name="Vaug")
            CCT = persist.tile([P, NP, S], FP8, name="CCTsb")

            PAIR_BAND = (0, 32, 64, 0)
            PAIR_SLOT = (0, 0, 0, 1)

            def shuffle_qk(dst, stg, pp, cols, split=False):
                # partition bands of the eviction staging -> the pair's band:
                # stg parts 32b.. = (head-in-pair b//2, d-half b%2).
                # split=True gens bands 2-3 on Pool SWDGE, halving the
                # serialized HWDGE generation on the startup-critical path
                pb, psl = PAIR_BAND[pp], PAIR_SLOT[pp]
                for b in range(4):
                    eng = nc.gpsimd if (split and b >= 2) else nc.sync
                    eng.dma_start(
                        out=dst[pb : pb + 32, psl, b // 2, b % 2, cols],
                        in_=stg[32 * b : 32 * (b + 1), :],
                    )

            # DoubleRow projection: 2 passes of K=256 (et-tile pairs)
            def dr_proj(pq_slice, wsb, w0, w1, cols):
                for j in range(2):
                    nc.tensor.matmul(
                        pq_slice,
                        lhsT=wsb[:, 2 * j : 2 * j + 2, w0:w1],
                        rhs=XT[:, 2 * j : 2 * j + 2, cols],
                        start=(j == 0),
                        stop=(j == 1),
                        perf_mode=PM.DoubleRow,
                    )

            # ---------------- stage 0: direct loads (host pre-layouts) -------
            with (
                tc.tile_pool(name="qkp", bufs=3, space="PSUM") as qkp,
            ):
                nc.vector.memset(Vaug[:, :, :, D : D + 1], 1.0)
                nc.vector.memset(ones8, 1.0)
                nc.vector.memset(ones_bf, 1.0)
                nc.vector.memset(ones_f32, 1.0)

                # PE warmup during the initial DMA wait: HAM un-throttles
                # after ~3.4us of sustained activity, so the first real
                # matmuls run at full clock instead of 1/2
                warm = qkp.tile([P, 1024], F32, tag="pq", name="warm")
                for _w in range(110):
                    nc.tensor.matmul(
                        warm[:, 0:64], lhsT=ones_bf, rhs=ones_bf[:, 0:64],
                        start=True, stop=True,
                    )
                # ACT table preload during the same wait: the first real Exp
                # otherwise pays the 1283ns table load on the critical path
                warm_act = persist.tile([1, 1], F32, name="warm_act")
                nc.scalar.activation(
                    out=warm_act, in_=ones_f32[:, 0:1], func=AF.Exp
                )

                # critical-chain DMA order, XT in column-halves: the first
                # scores need only seq columns 0-1023 (Q cc0 + K cc0), so
                # those halves + Wq/Wk gate the chain; cc1 columns follow
                nc.sync.dma_start(out=XT[:, 0, 0:1024], in_=xTD[0:P, 0:1024])
                nc.gpsimd.dma_start(
                    out=XT[:, 1, 0:1024], in_=xTD[P : 2 * P, 0:1024]
                )
                nc.sync.dma_start(
                    out=Wq_sb,
                    in_=wqpD[:].rearrange("(et p) hd -> p et hd", p=P),
                )
                nc.sync.dma_start(
                    out=Wk_sb,
                    in_=wkpD[:].rearrange("(et p) hd -> p et hd", p=P),
                )
                # XT2a/XT3a + bqk gens on the idle Pool SWDGE channel:
                # the serialized SP HWDGE generations pace the startup
                for et in range(2, ET):
                    nc.gpsimd.dma_start(
                        out=XT[:, et, 0:1024],
                        in_=xTD[et * P : (et + 1) * P, 0:1024],
                    )
                nc.sync.dma_start(out=bqk, in_=bqkD[:])
                tc.cur_priority += 100
                for et in range(ET):
                    nc.sync.dma_start(
                        out=XT[:, et, 1024:2048],
                        in_=xTD[et * P : (et + 1) * P, 1024:2048],
                    )
                tc.cur_priority -= 100

                # pair-0 Q cc0 + K cc0 first (gate the first scores), K cc1
                # after its XT columns land
                k0stg = persist.tile([P, S], FP8, name="k0stg")
                q0stg = persist.tile([P, 1024], FP8, name="q0stg")
                p0chunks = []
                for qk, cc2 in ((0, 0), (1, 0), (1, 1)):
                    pq = qkp.tile([P, 1024], F32, tag="pq", name="pq0")
                    p0chunks.append((qk, cc2, pq))

                def p0_mm(qk, cc2, pq, j):
                    wsb = Wq_sb if qk == 0 else Wk_sb
                    for c in range(2):
                        nc.tensor.matmul(
                            pq[:, c * 512 : (c + 1) * 512],
                            lhsT=wsb[:, 2 * j : 2 * j + 2, 0 : 2 * D],
                            rhs=XT[
                                :,
                                2 * j : 2 * j + 2,
                                (2 * cc2 + c) * 512 : (2 * cc2 + c + 1) * 512,
                            ],
                            start=(j == 0),
                            stop=(j == 1),
                            perf_mode=PM.DoubleRow,
                        )

                def p0_fin(i, pieces=(slice(0, 1024),)):
                    # piecewise: the first piece's shuffle gates the first
                    # score, so small leading pieces cut the startup chain
                    qk, cc2, pq = p0chunks[i]
                    base = cc2 * 1024
                    for pc in pieces:
                        if qk == 0:
                            dst = q0stg[:, pc]
                        else:
                            dst = k0stg[:, base + pc.start : base + pc.stop]
                        if i % 2 == 0:
                            nc.scalar.activation(
                                out=dst, in_=pq[:, pc], func=AF.Identity,
                                bias=bqk[:, qk, 0:1],
                            )
                        else:
                            nc.vector.tensor_scalar_add(
                                dst, pq[:, pc], bqk[:, qk, 0:1]
                            )
                        if qk == 0:
                            shuffle_qk(QTd, q0stg[:, pc], 0, pc, split=True)
                        else:
                            shuffle_qk(
                                KTd,
                                k0stg[:, base + pc.start : base + pc.stop],
                                0,
                                slice(base + pc.start, base + pc.stop),
                                split=True,
                            )

                for j in range(2):
                    p0_mm(*p0chunks[0], j)
                    p0_mm(*p0chunks[1], j)
                tc.cur_priority -= 5000
                p0_fin(0)
                p0_fin(1, (slice(0, 128), slice(128, 1024)))
                tc.cur_priority += 5000
                for j in range(2):
                    p0_mm(*p0chunks[2], j)
                p0_fin(2)

                # Wv right behind the critical chain (priority between
                # bqk and the XT cc1 halves): the V projections then retire
                # into the idle startup window instead of competing with
                # block-0/1 exps
                tc.cur_priority += 75
                nc.sync.dma_start(
                    out=Wv_sb,
                    in_=wvpD[:].rearrange("(et p) hd -> p et hd", p=P),
                )
                tc.cur_priority -= 75
                nc.gpsimd.dma_start(out=bv8, in_=bv8D[:])

                # stage-3 constants + residual input are loaded from the
                # chunk schedule (block 0 onward): emitting them here would
                # head-block the startup-critical shuffle DMAs on the shared
                # DMA engines

                def bulk_loads():
                    # fine-grained (1-st) chunks: the Tile scheduler hoists
                    # dep-free DMAs regardless of priority, so these WILL
                    # interleave with the startup-critical shuffles on the
                    # shared DMA engines — small pieces bound the head-block
                    # to ~0.7us
                    tc.cur_priority += 20000
                    nc.sync.dma_start(
                        out=Wo_sb,
                        in_=wopD[:].rearrange("(kt p) e -> p kt e", p=P),
                    )
                    if not identity_ln:
                        for dram, sb in ((gammaD, gamma_bc), (betaD, beta_bc)):
                            nc.sync.dma_start(out=sb, in_=_bcast_ap(dram[:], P))
                    # residual: sts 0-7 f32 (DVE-add path), sts 8-15 bf16
                    # (stage-3 eye-matmul path)
                    xDr = xD[:].rearrange("(st p) e -> p st e", p=P)
                    for q in range(8):
                        nc.sync.dma_start(
                            out=X[:, q : q + 1], in_=xDr[:, q : q + 1]
                        )
                    xbfr = xbfD[:].rearrange("(st p) e -> p st e", p=P)
                    for q in range(4):
                        nc.sync.dma_start(
                            out=Xbf[:, 2 * q : 2 * q + 2],
                            in_=xbfr[:, 8 + 2 * q : 10 + 2 * q],
                        )
                    nc.sync.dma_start(out=eye_bf, in_=eyeD[:])
                    tc.cur_priority -= 20000

            # ---------------- stage 2: attention ----------------
            with (
                tc.tile_pool(name="expp", bufs=9) as expp,
                tc.tile_pool(name="scp", bufs=4, space="PSUM") as scp,
                tc.tile_pool(name="smallp", bufs=3) as smallp,
                tc.tile_pool(name="cxsp", bufs=3) as cxsp,
                tc.tile_pool(name="outp", bufs=3) as outp,
                tc.tile_pool(name="statp", bufs=4) as statp,
            ):
                evict_flip = [1]

                def evict(dst, src, bias_ap=None, boost=0):
                    # PSUM->SBUF eviction, alternating ACT/DVE to balance
                    tc.cur_priority -= boost
                    evict_flip[0] ^= 1
                    if evict_flip[0]:
                        if bias_ap is None:
                            nc.scalar.activation(out=dst, in_=src, func=AF.Copy)
                        else:
                            nc.scalar.activation(
                                out=dst, in_=src, func=AF.Identity, bias=bias_ap
                            )
                    else:
                        if bias_ap is None:
                            nc.vector.tensor_copy(out=dst, in_=src)
                        else:
                            nc.vector.tensor_scalar_add(dst, src, bias_ap)
                    tc.cur_priority += boost

                # deferred work, interleaved through the scores PSUM slots
                def v_chunk(q):
                    def emit():
                        pv = scp.tile([P, 1024], F32, tag="SC", name="pv")
                        for c in range(2):
                            st = 2 * q + c
                            sl = pv[:, c * 512 : (c + 1) * 512]
                            for j in range(2):
                                nc.tensor.matmul(
                                    sl,
                                    lhsT=XT[:, 2 * j : 2 * j + 2, st * P : (st + 1) * P],
                                    rhs=Wv_sb[:, 2 * j : 2 * j + 2, :],
                                    start=(j == 0),
                                    stop=False,
                                    perf_mode=PM.DoubleRow,
                                )
                            nc.tensor.matmul(
                                sl, lhsT=ones8, rhs=bv8, start=False, stop=True
                            )
                        evict(
                            Vaug[:, 2 * q : 2 * q + 2, :, 0:D],
                            pv[:].rearrange("p (a h d) -> p a h d", a=2, h=H),
                            boost=1500,
                        )

                    return emit

                kstgs = {}

                def qk_chunk(pp, qk, cc2):
                    def emit():
                        wsb = Wq_sb if qk == 0 else Wk_sb
                        pq = scp.tile([P, 1024], F32, tag="SC", name="pq2")
                        for c in range(2):
                            dr_proj(
                                pq[:, c * 512 : (c + 1) * 512],
                                wsb,
                                2 * pp * D,
                                (2 * pp + 2) * D,
                                slice((2 * cc2 + c) * 512, (2 * cc2 + c + 1) * 512),
                            )
                        if qk == 0:
                            qstg = cxsp.tile([P, 1024], FP8, tag="qstg", name="qstg")
                            evict(qstg, pq, bqk[:, qk, pp : pp + 1], boost=1500)
                            shuffle_qk(
                                QTd, qstg, pp,
                                slice(cc2 * 1024, (cc2 + 1) * 1024),
                                split=True,
                            )
                        else:
                            # K both halves batched into one [P,S] staging so
                            # the shuffle is 4 full-row DMAs per pair
                            if pp not in kstgs:
                                kstgs[pp] = cxsp.tile(
                                    [P, S], FP8, tag="kstg", name="kstg"
                                )
                            evict(
                                kstgs[pp][:, cc2 * 1024 : (cc2 + 1) * 1024],
                                pq,
                                bqk[:, qk, pp : pp + 1],
                                boost=1500,
                            )
                            if cc2 == 1:
                                shuffle_qk(
                                    KTd, kstgs.pop(pp), pp, slice(0, S),
                                    split=True,
                                )

                    return emit

                def st3_chunk(st):
                    def emit():
                        tc.cur_priority += 3000
                        po = scp.tile([P, 1024], F32, tag="SC", name="po3")
                        for j in range(2):
                            nc.tensor.matmul(
                                po[:, 0:E],
                                lhsT=CCT[:, 2 * j : 2 * j + 2, st * P : (st + 1) * P],
                                rhs=Wo_sb[:, 2 * j : 2 * j + 2, :],
                                start=(j == 0),
                                stop=(j == 1),
                                perf_mode=PM.DoubleRow,
                            )
                        # DVE residual add doubles as the PSUM eviction (frees
                        # the borrowed score slot fast, no PE camp)
                        y = outp.tile([P, E], F32, tag="y", name="y")
                        nc.vector.tensor_add(y, po[:, 0:E], X[:, st])
                        stats = statp.tile([P, 6], F32, tag="stats", name="stats")
                        nc.vector.bn_stats(out=stats, in_=y)
                        mv = statp.tile([P, 2], F32, tag="mv", name="mv")
                        nc.vector.bn_aggr(out=mv, in_=stats)
                        rstd = statp.tile([P, 1], F32, tag="rstd", name="rstd")
                        # rstd = exp(-0.5*ln(var+eps)): Ln and Exp share one
                        # ACT table set with the softmax exps
                        nc.scalar.activation(
                            out=rstd, in_=mv[:, 1:2], func=AF.Ln, bias=eps_t
                        )
                        nc.scalar.activation(
                            out=rstd, in_=rstd, func=AF.Exp, scale=-0.5
                        )
                        yb = outp.tile([P, E], BF16, tag="yb", name="yb")
                        if identity_ln:
                            nc.gpsimd.tensor_scalar(
                                yb, y, mv[:, 0:1], rstd, OP.subtract, OP.mult
                            )
                        else:
                            nc.gpsimd.tensor_scalar(
                                y, y, mv[:, 0:1], rstd, OP.subtract, OP.mult
                            )
                            nc.gpsimd.tensor_tensor(y, y, gamma_bc, OP.mult)
                            nc.gpsimd.tensor_tensor(yb, y, beta_bc, OP.add)
                        nc.sync.dma_start(out=outD[st * P : (st + 1) * P, :], in_=yb)
                        tc.cur_priority -= 3000

                    return emit

                Q, K = 0, 1
                # chunk schedule over 16 (sh, h) blocks: pair p's Q/K due at
                # blk 2p (sh0); Q cc2=1 due at blk 8+2p (sh1); st3(st<8) after
                # blk 7 completes CCT's sh0 columns
                sched = {
                    0: [(t, v_chunk(3 + t // 2)) for t in range(1, 10, 2)]
                    + [(10, v_chunk(1)), (11, v_chunk(2)), (5, qk_chunk(1, K, 0)),
                       (9, qk_chunk(1, Q, 0)), (13, qk_chunk(1, K, 1)),
                       (15, bulk_loads)],
                    1: [(4, qk_chunk(2, K, 0)), (8, qk_chunk(2, Q, 0)),
                        (12, qk_chunk(2, K, 1))],
                    3: [(4, qk_chunk(3, K, 0)), (8, qk_chunk(3, Q, 0)),
                        (12, qk_chunk(3, K, 1))],
                    5: [(4, qk_chunk(0, Q, 1))],
                    6: [(4, qk_chunk(1, Q, 1))],
                    7: [(4, qk_chunk(2, Q, 1))],
                    8: [(4, qk_chunk(3, Q, 1))],
                    9: [(4, st3_chunk(0))],
                    10: [(4, st3_chunk(1))],
                    11: [(4, st3_chunk(2))],
                    12: [(4, st3_chunk(3))],
                    13: [(4, st3_chunk(4))],
                    14: [(4, st3_chunk(5)), (10, st3_chunk(6))],
                    15: [(4, st3_chunk(7))],
                }

                eps_t = statp.tile([P, 1], F32, tag="eps", bufs=1)
                nc.vector.memset(eps_t, LN_EPS)

                pending = []

                def block_tail(ets, h, pp, hl, s0, blk):
                    # ctx matmuls in three tranches (t1/t2/t3 of the next
                    # block) so PE's in-order queue never camps >0.5us on the
                    # burst before the next block's score matmuls; the
                    # accumulator BORROWS a score slot (tag SC) until the
                    # normalize (t4) evicts it. Block 14 (hl1, tail-critical)
                    # splits its ctx: den row at partition 0, V rows at
                    # 64-127, so its normalize is partition-aligned for the
                    # fast (bounce-free) path.
                    cell = {}

                    def emit_pairs(lo, hi):
                        def emit():
                            if "cx" not in cell:
                                cell["cx"] = scp.tile(
                                    [D + 1, 1024], F32, tag="SC", name="cx"
                                )
                            for tp in range(lo, hi):
                                ctx_pair(cell["cx"], ets, h, tp, blk)

                        return emit

                    def emit_norm():
                        cx = cell["cx"]
                        # normalize: row D of cx is the softmax denominator.
                        # evict to SBUF; Pool broadcasts the den row across
                        # partitions, bit-trick reciprocal, multiply.
                        # The LAST TWO blocks are priority-boosted: the
                        # stage-3 tail critical path runs through their
                        # normalizes.
                        tc.cur_priority -= 2000
                        cxs = cxsp.tile([D + 1, 1024], F32, tag="cxs", name="cxs")
                        if blk == 15:
                            # ACT evict so DVE starts the recip immediately
                            nc.scalar.activation(out=cxs, in_=cx, func=AF.Copy)
                        else:
                            evict(cxs, cx)
                        if blk == 15:
                            # tail-critical: skip the DRAM bounce — DVE
                            # recips the den row straight out of PSUM (bf16,
                            # so the idle-PE broadcast matmul runs 1cyc/col),
                            # DVE multiplies (ends on an hl0 head by block
                            # order). Only ONE STT input may be PSUM (dps).
                            rrow = smallp.tile([1, 1024], BF16, tag="rrow", name="rr")
                            with nc.allow_low_precision(
                                reason="bf16 recip row; fp8 exps dominate err"
                            ):
                                nc.vector.reciprocal(rrow, cx[D : D + 1, :])
                            dps = scp.tile([P, 1024], F32, tag="SC", name="dps")
                            for cc in range(2):
                                nc.tensor.matmul(
                                    dps[0:D, cc * 512 : (cc + 1) * 512],
                                    lhsT=ones_bf[:, 0:D],
                                    rhs=rrow[:, cc * 512 : (cc + 1) * 512],
                                    start=True,
                                    stop=True,
                                )
                            nc.vector.scalar_tensor_tensor(
                                CCT[0:D, pp, s0 : s0 + 1024],
                                cxs[0:D, :],
                                0.125,
                                dps[0:D, 0:1024],
                                OP.mult,
                                OP.mult,
                            )
                            tc.cur_priority += 2000
                            return
                        dden = dramp.tile([1, 1024], F32, tag="dden", name="dden")
                        nc.sync.dma_start(out=dden, in_=cxs[D : D + 1, :])
                        dbc = smallp.tile([D, 1024], F32, tag="dbc", name="dbc")
                        nc.sync.dma_start(out=dbc, in_=_bcast_ap(dden[0], D))
                        rec = smallp.tile([D, 1024], F32, tag="rec", name="rec")
                        nc.gpsimd.tensor_scalar(
                            rec[:].bitcast(I32), dbc[:].bitcast(I32),
                            -1, REC_C, OP.mult, OP.add,
                        )
                        if hl == 0:
                            nc.gpsimd.tensor_tensor(
                                CCT[0:D, pp, s0 : s0 + 1024], cxs[0:D, :], rec,
                                OP.mult,
                            )
                        else:
                            # result lands on partitions 64..127; Pool cannot
                            # shift partitions, DMA can.
                            tmp = smallp.tile([D, 1024], FP8, tag="tmp", name="tmp")
                            nc.gpsimd.tensor_tensor(tmp, cxs[0:D, :], rec, OP.mult)
                            nc.sync.dma_start(
                                out=CCT[D : 2 * D, pp, s0 : s0 + 1024], in_=tmp
                            )
                        tc.cur_priority += 2000 if blk == 15 else 0

                    return [
                        emit_pairs(0, 2),
                        emit_pairs(2, 5),
                        emit_pairs(5, 8),
                        emit_norm,
                    ]

                def ctx_pair(cx, ets, h, tp, blk):
                    et_p = ets.pop(tp)
                    for cc in range(2):
                        nc.tensor.matmul(
                            cx[:, cc * 512 : (cc + 1) * 512],
                            lhsT=Vaug[:, 2 * tp : 2 * tp + 2, h, 0 : D + 1],
                            rhs=et_p[:, :, cc * 512 : (cc + 1) * 512],
                            start=(tp == 0),
                            stop=(tp == ST // 2 - 1),
                            perf_mode=PM.DoubleRow,
                        )

                # sh1 head order ends on an hl0 head: the last block's CCT
                # write then skips the partition-shift DMA hop
                for sh, horder in ((0, range(H)), (1, (1, 0, 3, 2, 5, 4, 7, 6))):
                    for h in horder:
                        s0 = sh * 1024
                        pp, hl = h // 2, h % 2
                        pb, psl = PAIR_BAND[pp], PAIR_SLOT[pp]
                        blk = sh * H + (h if sh == 0 else {1:0,0:1,3:2,2:3,5:4,4:5,7:6,6:7}[h])
                        slots = {}
                        for t, fn in sched.get(blk, []):
                            slots.setdefault(t, []).append(fn)

                        if blk == 0:
                            v_chunk(0)()
                        ets = {}

                        for t in range(ST):
                            for fn in slots.get(t, []):
                                fn()
                            sc = scp.tile([P, 1024], F32, tag="SC", name="sc")
                            for cc in range(2):
                                if blk == 0 and t < 4:
                                    # first scores straight off the eviction
                                    # staging (h0 dims on partitions 0-63):
                                    # plain k=64 fp8 matmul skips the ~3us
                                    # band-shuffle wait; PE is idle here so
                                    # the 2x matmul cost is free
                                    nc.tensor.matmul(
                                        sc[:, cc * 512 : (cc + 1) * 512],
                                        lhsT=k0stg[0:64, t * P : (t + 1) * P],
                                        rhs=q0stg[
                                            0:64, cc * 512 : (cc + 1) * 512
                                        ],
                                        start=True,
                                        stop=True,
                                    )
                                    continue
                                nc.tensor.matmul(
                                    sc[:, cc * 512 : (cc + 1) * 512],
                                    lhsT=KTd[
                                        pb : pb + 32,
                                        psl,
                                        h % 2,
                                        :,
                                        t * P : (t + 1) * P,
                                    ],
                                    rhs=QTd[
                                        pb : pb + 32,
                                        psl,
                                        h % 2,
                                        :,
                                        s0 + cc * 512 : s0 + (cc + 1) * 512,
                                    ],
                                    start=True,
                                    stop=True,
                                    perf_mode=PM.DoubleRow,
                                )
                            if t % 2 == 0:
                                ets[t // 2] = expp.tile(
                                    [P, 2, 1024], FP8, tag="expT", name="et_t"
                                )
                            dst = ets[t // 2][:, t % 2, :]
                            if t in (ACT_TS10 if blk in ACT10_BLOCKS else ACT_TS9):
                                nc.scalar.activation(
                                    out=dst, in_=sc, func=AF.Exp, scale=EXP_SCALE
                                )
                            else:
                                nc.vector.tensor_scalar(
                                    dst.bitcast(I8), sc, SCH_A, SCH_B,
                                    OP.mult, OP.add,
                                )
                            # previous block's tail (ctx tranches +
                            # normalize) drains at t1..t4, one piece per t,
                            # so the boundary score matmuls and the freshly
                            # freed score slots are never behind it
                            if pending:
                                pending.pop(0)()
                            # last block: drain its own ctx tranches at
                            # t13/t14/t15 (each needs only already-emitted
                            # exps) so just the normalize remains after the
                            # loop
                            if blk == 15 and t == 12:
                                pending[:] = block_tail(ets, h, pp, hl, s0, blk)
                        if blk < 15:
                            pending[:] = block_tail(ets, h, pp, hl, s0, blk)
                # flush the final block's normalize
                for fn in pending:
                    fn()

            # ---------------- stage 3: Wo, residual, LayerNorm ----------------
            with (
                tc.tile_pool(name="outp3", bufs=6) as outp3,
                tc.tile_pool(name="ps3", bufs=8, space="PSUM") as ps3,
                tc.tile_pool(name="statp3", bufs=8) as statp3,
            ):
                eps_t = statp3.tile([P, 1], F32, tag="eps", bufs=1, name="eps_t3")
                nc.vector.memset(eps_t, LN_EPS)
                yb2s = [None]
                # deprioritized: fills engine-idle slots during the last
                # attention block instead of starving its scores
                tc.cur_priority += 20000
                # two passes: all stat chains first (the tiny ACT rstds must
                # not queue behind 612ns evicts), then the evicts/stores
                chains = []
                for st in range(8, ST):
                    po = ps3.tile([P, E], F32, tag="po", name="po")
                    for j in range(2):
                        nc.tensor.matmul(
                            po,
                            lhsT=CCT[:, 2 * j : 2 * j + 2, st * P : (st + 1) * P],
                            rhs=Wo_sb[:, 2 * j : 2 * j + 2, :],
                            start=(j == 0),
                            stop=False,
                            perf_mode=PM.DoubleRow,
                        )
                    # residual add on otherwise-idle PE: po += I.T @ (x+bo),
                    # bf16 so the 8-st tail chain stays short on PE
                    nc.tensor.matmul(
                        po, lhsT=eye_bf, rhs=Xbf[:, st - 8], start=False, stop=True
                    )
                    stats = statp3.tile([P, 6], F32, tag="stats", name="stats")
                    nc.vector.bn_stats(out=stats, in_=po)
                    mv = statp3.tile([P, 2], F32, tag="mv", name="mv")
                    nc.vector.bn_aggr(out=mv, in_=stats)
                    rstd = statp3.tile([P, 1], F32, tag="rstd", name="rstd")
                    nc.scalar.activation(
                        out=rstd, in_=mv[:, 1:2], func=AF.Ln, bias=eps_t
                    )
                    nc.scalar.activation(out=rstd, in_=rstd, func=AF.Exp, scale=-0.5)
                    nm = statp3.tile([P, 1], F32, tag="nm", name="nm")
                    nc.vector.tensor_scalar(nm, mv[:, 0:1], rstd, -1.0, OP.mult, OP.mult)
                    chains.append((st, po, rstd, nm))
                for st, po, rstd, nm in chains:
                    if st % 2 == 0:
                        yb2 = outp3.tile([P, 2, E], BF16, tag="yb", name="yb")
                        yb2s[0] = yb2
                    else:
                        yb2 = yb2s[0]
                    ybs = yb2[:, st % 2, :]
                    if identity_ln:
                        # st15 on DVE (its bn chain drains just before): the
                        # evicts otherwise pace the kernel tail on ACT alone
                        if st < 15:
                            nc.scalar.activation(
                                out=ybs, in_=po, func=AF.Identity, bias=nm,
                                scale=rstd,
                            )
                        else:
                            nc.vector.tensor_scalar(
                                ybs, po, rstd, nm, OP.mult, OP.add
                            )
                    else:
                        y = outp3.tile([P, E], F32, tag="y", name="y")
                        nc.scalar.activation(
                            out=y, in_=po, func=AF.Identity, bias=nm, scale=rstd
                        )
                        if st % 2 == 0:
                            nc.gpsimd.tensor_tensor(y, y, gamma_bc, OP.mult)
                            nc.vector.tensor_tensor(ybs, y, beta_bc, OP.add)
                        else:
                            nc.vector.tensor_tensor(y, y, gamma_bc, OP.mult)
                            nc.gpsimd.tensor_tensor(ybs, y, beta_bc, OP.add)
                    if st % 2 == 1:
                        # one gen per two sts: the out-DMA HWDGE generations
                        # were pacing the kernel tail
                        nc.sync.dma_start(
                            out=outD[(st - 1) * P : (st + 1) * P, :].rearrange(
                                "(q p) e -> p q e", p=P
                            ),
                            in_=yb2,
                        )
                tc.cur_priority -= 20000

    _patch_to_json(nc)
    return nc


_NC_CACHE = {}


def _get_nc(identity_ln):
    if identity_ln not in _NC_CACHE:
        _NC_CACHE[identity_ln] = build_nc(identity_ln)
    return _NC_CACHE[identity_ln]


def kernel(**inputs) -> np.ndarray:
    import ml_dtypes
    from concourse.bass_utils import run_bass_kernel_spmd

    F8 = ml_dtypes.float8_e4m3fn
    gamma = np.asarray(inputs["gamma"], np.float32)
    beta = np.asarray(inputs["beta"], np.float32)
    # LN gamma/beta are identity in the common case — drop their
    # tensor-tensor ops from the build when so (general path kept as
    # fallback for arbitrary gamma/beta)
    identity_ln = bool(np.all(gamma == 1.0) and np.all(beta == 0.0))
    nc = _get_nc(identity_ln)
    x = np.asarray(inputs["x"], dtype=np.float32)
    B = x.shape[0]

    def perm_w8(k):  # [H, E, D] -> [E, H*D] fp8, x8 scale
        w = np.asarray(inputs[k], dtype=np.float32) * 8.0
        return np.ascontiguousarray(w.transpose(1, 0, 2).reshape(E, H * D).astype(F8))

    bqk = np.ascontiguousarray(
        np.stack(
            [
                np.asarray(inputs["bq"], np.float32).reshape(NP, P).T * 8.0,
                np.asarray(inputs["bk"], np.float32).reshape(NP, P).T * 8.0,
            ],
            axis=1,
        )
    )
    shared = {
        "Wq_p": perm_w8("Wq"),
        "Wk_p": perm_w8("Wk"),
        "Wv_p": perm_w8("Wv"),
        # CCT holds ctx_true (the 1/(8 den) is folded into the bit-trick
        # reciprocal), so Wo ships unscaled
        "Wo_p": np.ascontiguousarray(np.asarray(inputs["Wo"], np.float32).astype(F8)),
        "bqk": bqk,
        "bv8": np.ascontiguousarray(
            (np.asarray(inputs["bv"], np.float32) * 8.0).reshape(1, H * D).astype(F8)
        ),
        "eye": np.ascontiguousarray(np.eye(P, dtype=ml_dtypes.bfloat16)),
        "gamma": np.ascontiguousarray(np.asarray(inputs["gamma"], np.float32)),
        "beta": np.ascontiguousarray(np.asarray(inputs["beta"], np.float32)),
    }
    bo = np.asarray(inputs["bo"], np.float32)
    in_maps = []
    for b in range(B):
        xb = np.ascontiguousarray(x[b])
        xpb = np.ascontiguousarray(xb + bo)
        in_maps.append(
            {
                "xpb": xpb,
                "xpb_bf": np.ascontiguousarray(xpb.astype(ml_dtypes.bfloat16)),
                "xT": np.ascontiguousarray(xb.T.astype(F8)),
                **shared,
            }
        )
    res = run_bass_kernel_spmd(nc, in_maps, core_ids=list(range(B)))
    return np.stack(
        [res.results[b]["out"].astype(np.float32) for b in range(B)], axis=0
    )

